# revision 1
# baseline (speedup 1.0000x reference)
"""GraphSAGE 2-layer GNN on TRN2, 8-core SPMD Bass/Tile kernel (v2).

Strategy:
- Nodes sharded across 8 cores (6250 each). Edge slots sorted by
  (dst tile, src parity), padded to 128-slot chunks with a per-(tile,parity)
  budget equal across cores (SPMD uniform).
- Layer 1 messages are HOST-STAGED: x[src] in slot order, pre-swizzled to the
  SBUF window layout, streamed with plain HWDGE dma_start (no gpsimd descgen).
- Segment-sum via one-hot matmul on PE. One-hots are HOST-PRECOMPUTED fp8
  constants resident in SBUF (no DVE is_equal). One-hot columns use a
  parity permutation: partitions 0:64 = even-local nodes, 64:128 = odd.
- Layer 2: z = h @ Wl2 (40 cols padded to 64, bf16) packed in node PAIRS:
  z2 row r = [z[2r] | z[2r+1]] (256B rows). AllGather of z2 (6.4MB) is
  CHUNKED over tile ranges to overlap layer-1 tails. dma_gather (win=1024,
  the ucode max) fetches z2 rows; chunk parity selects column half.
- Per-tile PSUM scale/copies run on the Activation engine (per-partition
  invc scale AP); PE does transposes + dense matmuls in bf16.
"""
from dataclasses import dataclass, field
import numpy as np
import ml_dtypes

import concourse.bacc as bacc
import concourse.bass as bass
import concourse.mybir as mybir
import concourse.tile as tile
from concourse import library_config

P = 128
FP8 = ml_dtypes.float8_e4m3
BF16 = ml_dtypes.bfloat16


@dataclass
class Plan:
    n_nodes: int
    n_feat: int
    n_hid: int
    n_class: int
    n_cores: int
    npc: int                 # nodes per core
    nt: int                  # dst tiles per core
    nhr: int                 # z2 pair-rows per core (nt*64)
    win: int                 # L2 dma_gather window (slots)
    win1: int                # L1 dma_start window (slots)
    regions: list            # AG region boundaries in tiles, e.g. [28, 42, 49]
    budget: np.ndarray       # [nt, nregions*2] chunks per (tile, group)
    nch: int = 0             # total chunks per core per layer
    S: int = 0               # total slots (nch*128)
    # per-global-chunk metadata
    chunk_par: np.ndarray = None    # [nch] parity
    chunk_reg: np.ndarray = None    # [nch] region
    chunk_sci: np.ndarray = None    # [nch] index within region stream
    SR: list = field(default_factory=list)         # slots per region stream
    # per-core staged constant arrays
    oh_tab: list = field(default_factory=list)     # [128, nch, 128] fp8
    idxR: list = field(default_factory=list)       # per core: list per region
    src_slot: list = field(default_factory=list)   # [S] int64 (-1 pad)
    invc_perm: list = field(default_factory=list)  # [128, nt] f32


def _wrap_idx(arr_i16: np.ndarray) -> np.ndarray:
    # position j -> partition j%16, col j//16; replicated 8x down partitions
    w = arr_i16.reshape(-1, 16).T            # [16, n/16]
    return np.ascontiguousarray(np.tile(w, (8, 1)))  # [128, n/16]


def make_plan(edge_index: np.ndarray, n_nodes: int, n_feat: int, n_hid: int,
              n_class: int, n_cores: int, win: int = 1024, win1: int = 4096,
              regions: list | None = None) -> Plan:
    src = np.asarray(edge_index[0], dtype=np.int64)
    dst = np.asarray(edge_index[1], dtype=np.int64)
    npc = n_nodes // n_cores
    assert npc * n_cores == n_nodes and npc % 2 == 0
    nt = (npc + P - 1) // P
    nhr = nt * 64

    deg = np.bincount(dst, minlength=n_nodes).astype(np.float64)
    invc = (1.0 / np.maximum(deg, 1.0)).astype(np.float32)

    core_of = dst // npc
    rem = dst - core_of * npc
    tloc = rem // P
    loc = rem - tloc * P
    dp = (loc >> 1) + 64 * (loc & 1)           # permuted one-hot column
    par = (src & 1).astype(np.int64)            # parity of src (npc even)

    # AG region boundaries (in dst tiles of the SRC core's z2 rows).
    # A single region measured fastest (AG is cheap; padding isn't).
    if regions is None:
        regions = [nt]
    assert regions[-1] == nt
    nreg = len(regions)
    rb_tiles = np.array([0] + regions)          # region tile boundaries
    rb_rows = rb_tiles * 64                     # z2-row boundaries per core

    src_row = (src % npc) >> 1                  # z2 row within src core
    src_reg = np.searchsorted(rb_rows[1:], src_row, side="right")
    grp = src_reg * 2 + par                     # group id per edge
    ng = nreg * 2

    counts = np.zeros((n_cores, nt, ng), dtype=np.int64)
    np.add.at(counts, (core_of, tloc, grp), 1)
    budget = np.ceil(counts.max(axis=0) / P).astype(np.int64)  # [nt, ng]
    nch = int(budget.sum())
    S = nch * P

    plan = Plan(n_nodes=n_nodes, n_feat=n_feat, n_hid=n_hid, n_class=n_class,
                n_cores=n_cores, npc=npc, nt=nt, nhr=nhr, win=win, win1=win1,
                regions=list(regions), budget=budget, nch=nch, S=S)

    # chunk offsets per (t, g) in chunk units (global chunk order)
    flat = budget.reshape(-1)
    chunk_off = np.concatenate([[0], np.cumsum(flat)])[:-1].reshape(nt, ng)
    chunk_par = np.zeros(nch, np.int8)
    chunk_reg = np.zeros(nch, np.int8)
    chunk_sci = np.zeros(nch, np.int64)
    sci = [0] * nreg
    for t in range(nt):
        for g in range(ng):
            r, p = g // 2, g % 2
            o = chunk_off[t, g]
            nb = int(budget[t, g])
            chunk_par[o:o + nb] = p
            chunk_reg[o:o + nb] = r
            chunk_sci[o:o + nb] = np.arange(sci[r], sci[r] + nb)
            sci[r] += nb
    plan.chunk_par, plan.chunk_reg, plan.chunk_sci = chunk_par, chunk_reg, chunk_sci
    plan.SR = [int(s) * P for s in sci]

    # sort edges by (core, tile, group)
    key = core_of * (nt * ng) + tloc * ng + grp
    order = np.argsort(key, kind="stable")
    srcg = src[order]; keyg = key[order]
    dpg = dp[order]

    for c in range(n_cores):
        lo = np.searchsorted(keyg, c * nt * ng, side="left")
        hi = np.searchsorted(keyg, (c + 1) * nt * ng, side="left")
        sel = slice(lo, hi)
        st = srcg[sel]; kt = keyg[sel] - c * nt * ng; dt_ = dpg[sel]

        src_slot = np.full(S, -1, np.int64)
        dp_slot = np.full(S, -1, np.int64)
        bounds = np.concatenate([[0], np.where(np.diff(kt) != 0)[0] + 1, [len(st)]])
        for b0, b1 in zip(bounds[:-1], bounds[1:]):
            k = int(kt[b0]); t = k // ng; g = k % ng
            o = int(chunk_off[t, g]) * P
            n = b1 - b0
            src_slot[o:o + n] = st[b0:b1]
            dp_slot[o:o + n] = dt_[b0:b1]

        # one-hot table fp8: [128 slot-partitions, nch, 128]
        oh = np.zeros((P, nch, P), FP8)
        s_all = np.arange(S)
        valid = dp_slot >= 0
        oh[s_all[valid] % P, s_all[valid] // P, dp_slot[valid]] = 1.0
        plan.oh_tab.append(np.ascontiguousarray(oh))

        # L2 indices per region stream: row id within that region's table
        sv = np.where(src_slot >= 0, src_slot, 0)
        sc = sv // npc
        srow = (sv % npc) >> 1
        sreg = np.searchsorted(rb_rows[1:], srow, side="right")
        rrows = np.diff(rb_rows)                   # rows per region per core
        idx_in_reg = sc * rrows[sreg] + (srow - rb_rows[sreg])
        idxs = []
        for r in range(nreg):
            # slots of this region in global order
            chunks_r = np.where(chunk_reg == r)[0]
            slot_sel = (chunks_r[:, None] * P + np.arange(P)[None, :]).reshape(-1)
            vals = idx_in_reg[slot_sel]
            # padding slots inside these chunks point at row 0 of this region
            pad = src_slot[slot_sel] < 0
            vals = np.where(pad, 0, vals)
            assert vals.max() < 32768
            idxs.append(_wrap_idx(vals.astype(np.int16)))
        plan.idxR.append(idxs)
        plan.src_slot.append(src_slot)

        # permuted invc: partition p<64 -> loc 2p ; p>=64 -> loc 2(p-64)+1
        ic = np.zeros((P, nt), np.float32)
        base = c * npc
        for t in range(nt):
            rows = min(P, npc - t * P)
            locs = np.concatenate([np.arange(0, rows, 2), np.arange(1, rows, 2)])
            pos = np.concatenate([np.arange(0, (rows + 1) // 2),
                                  64 + np.arange(0, rows // 2)])
            ic[pos, t] = invc[base + t * P + locs]
        plan.invc_perm.append(ic)
    return plan


def stage_inputs(plan: Plan, x, Wl1, Wr1, b1, Wl2, Wr2, b2):
    """Build per-core in_maps (numpy) for the bass program."""
    n, f = x.shape
    hid = plan.n_hid
    ncl = plan.n_class
    npc, nt, S = plan.npc, plan.nt, plan.S
    win1 = plan.win1
    x_f32 = np.asarray(x, dtype=np.float32)
    x_bf = x_f32.astype(BF16)
    wl1 = np.asarray(Wl1, np.float32).astype(BF16)
    wr1 = np.asarray(Wr1, np.float32).astype(BF16)
    wl2p = np.zeros((hid, 64), BF16)
    wl2p[:, :ncl] = np.asarray(Wl2, np.float32).astype(BF16)
    wr2 = np.asarray(Wr2, np.float32).astype(BF16)
    b1c = np.asarray(b1, np.float32).reshape(hid, 1)
    b2bc = np.broadcast_to(np.asarray(b2, np.float32), (P, ncl)).copy()

    nw1 = (S + win1 - 1) // win1
    Sw = nw1 * win1

    in_maps = []
    for c in range(plan.n_cores):
        ss = plan.src_slot[c]
        msgs = np.zeros((Sw, f), BF16)
        valid = ss >= 0
        msgs[:S][valid] = x_bf[ss[valid]]
        # swizzle to SBUF window layout [nw1, 128, win1//128, f]
        msw = np.ascontiguousarray(
            msgs.reshape(nw1, win1 // P, P, f).transpose(0, 2, 1, 3))

        # permuted xT: [128, nt*128] column t*128+p = x[node(c,t,p)]
        xt = np.zeros((P, nt * P), BF16)
        base = c * npc
        for t in range(nt):
            rows = min(P, npc - t * P)
            locs = np.concatenate([np.arange(0, rows, 2), np.arange(1, rows, 2)])
            pos = np.concatenate([np.arange(0, (rows + 1) // 2),
                                  64 + np.arange(0, rows // 2)])
            xt[:, t * P + pos] = x_bf[base + t * P + locs].T

        im = {
            "msgs1": msw,
            "oh_tab": plan.oh_tab[c],
            "invc": plan.invc_perm[c],
            "xT_own": xt,
            "wl1": wl1, "wr1": wr1, "wl2p": wl2p, "wr2": wr2,
            "b1": b1c, "b2": b2bc, "ident": np.eye(P, dtype=np.float32),
        }
        for r in range(len(plan.regions)):
            im[f"idx_r{r}"] = plan.idxR[c][r]
        in_maps.append(im)
    return in_maps


def build_program(plan: Plan, repeats: int = 1, parts: str = "full",
                  single_packet: bool = True, m2_bufs: int | None = None):
    # parts: "l1" = layer1 only; "l1ag" = layer1 + allgathers; "full" = everything
    do_ag = parts in ("l1ag", "full")
    do_l2 = parts == "full"
    n = plan.n_nodes
    f = plan.n_feat
    hid = plan.n_hid
    ncl = plan.n_class
    nt = plan.nt
    npc = plan.npc
    nhr = plan.nhr
    nch = plan.nch
    S = plan.S
    win = plan.win
    win1 = plan.win1
    nw1 = (S + win1 - 1) // win1
    f32 = mybir.dt.float32
    bf16 = mybir.dt.bfloat16
    fp8 = mybir.dt.float8e4
    ncores = plan.n_cores
    budget = plan.budget

    nc = bacc.Bacc("TRN2", target_bir_lowering=False, debug=False,
                   enable_asserts=False, num_devices=ncores,
                   num_swdge_queues=4)

    regions = plan.regions
    nreg = len(regions)
    rb_tiles = [0] + regions
    rb_rows = [b * 64 for b in rb_tiles]
    SR = plan.SR
    chunk_par = plan.chunk_par
    chunk_reg = plan.chunk_reg
    chunk_sci = plan.chunk_sci

    msgs1_d = nc.dram_tensor("msgs1", [nw1, P, win1 // P, f], bf16, kind="ExternalInput")
    oh_d = nc.dram_tensor("oh_tab", [P, nch, P], fp8, kind="ExternalInput")
    idxR_d = [nc.dram_tensor(f"idx_r{r}", [P, SR[r] // 16], mybir.dt.int16,
                             kind="ExternalInput") for r in range(nreg)]
    invc_d = nc.dram_tensor("invc", [P, nt], f32, kind="ExternalInput")
    xT_d = nc.dram_tensor("xT_own", [P, nt * P], bf16, kind="ExternalInput")
    wl1_d = nc.dram_tensor("wl1", [f, hid], bf16, kind="ExternalInput")
    wr1_d = nc.dram_tensor("wr1", [f, hid], bf16, kind="ExternalInput")
    wl2p_d = nc.dram_tensor("wl2p", [hid, 64], bf16, kind="ExternalInput")
    wr2_d = nc.dram_tensor("wr2", [hid, ncl], bf16, kind="ExternalInput")
    b1_d = nc.dram_tensor("b1", [hid, 1], f32, kind="ExternalInput")
    b2_d = nc.dram_tensor("b2", [P, ncl], f32, kind="ExternalInput")
    ident_d = nc.dram_tensor("ident", [P, P], f32, kind="ExternalInput")
    out_d = nc.dram_tensor("out", [npc, ncl], f32, kind="ExternalOutput")

    if m2_bufs is None:
        m2_bufs = 8 if nreg == 1 else 4
    with tile.TileContext(nc) as tc:
        nc.gpsimd.load_library(library_config.mlp)
        with tc.tile_pool(name="const", bufs=1) as cp, \
             tc.tile_pool(name="store", bufs=1) as sp, \
             tc.tile_pool(name="m1", bufs=2) as mp1, \
             tc.tile_pool(name="m2", bufs=m2_bufs) as mp2, \
             tc.tile_pool(name="fin", bufs=2) as fp, \
             tc.tile_pool(name="seg", bufs=2, space="PSUM") as psum_seg, \
             tc.tile_pool(name="paux", bufs=3, space="PSUM") as psum_aux, \
             tc.tile_pool(name="phT", bufs=2, space="PSUM") as psum_h, \
             tc.tile_pool(name="dram", bufs=1, space="DRAM") as dp:

            # ---- constant staging ----
            def load_const(dram, shape, dtype=f32, tag="", slices=1):
                t = cp.tile(shape, dtype, tag=tag)
                if slices == 1:
                    nc.sync.dma_start(t[:], dram[:])
                else:
                    step = shape[1] // slices
                    for i in range(slices):
                        sl = slice(i * step, (i + 1) * step if i < slices - 1 else shape[1])
                        nc.sync.dma_start(t[:, sl], dram[:, sl])
                return t

            oh_t = load_const(oh_d, [P, nch, P], fp8, tag="oh", slices=8)
            idxR_t = [load_const(idxR_d[r], [P, SR[r] // 16], mybir.dt.int16,
                                 tag=f"idx_r{r}") for r in range(nreg)]
            invc_t = load_const(invc_d, [P, nt], tag="invc")
            xT_t = load_const(xT_d, [P, nt * P], bf16, tag="xT", slices=2)
            wl1_t = load_const(wl1_d, [f, hid], bf16, tag="wl1")
            wr1_t = load_const(wr1_d, [f, hid], bf16, tag="wr1")
            wl2p_t = load_const(wl2p_d, [hid, 64], bf16, tag="wl2p")
            wr2_t = load_const(wr2_d, [hid, ncl], bf16, tag="wr2")
            b1_t = load_const(b1_d, [hid, 1], tag="b1")
            b2_t = load_const(b2_d, [P, ncl], tag="b2")
            ident_t = load_const(ident_d, [P, P], tag="ident")

            hT_store = sp.tile([P, nt * P], bf16, tag="hT")     # [hid, node']
            z_acc = sp.tile([P, nt, 64], bf16, tag="z_acc")     # [node', tile, zcol]
            out_acc = sp.tile([P, nt, ncl], f32, tag="out_acc")

            for _rep in range(repeats):
                z2_own = dp.tile([nhr, P], bf16)
                z2_fullR = []
                for r in range(nreg):
                    zf = dp.tile([ncores, rb_rows[r + 1] - rb_rows[r], P], bf16,
                                 addr_space="Shared", name=f"z2full{r}")
                    z2_fullR.append(zf)

                # ================= LAYER 1 =================
                msg_bufs = {}

                def ensure_win1(w):
                    if w in msg_bufs:
                        return msg_bufs[w]
                    mt = mp1.tile([P, win1 // P, f], bf16, tag="m1")
                    nc.sync.dma_start(mt[:], msgs1_d[w, :, :, :])
                    msg_bufs[w] = mt
                    return mt

                ci = 0
                t_done = 0
                for t in range(nt):
                    ncht = int(budget[t, :].sum())
                    pt = psum_seg.tile([P, f], f32, tag="seg")
                    for j in range(ncht):
                        w, col = divmod(ci * P, win1)
                        mt = ensure_win1(w)
                        nc.tensor.matmul(out=pt[:], lhsT=oh_t[:, ci, :],
                                         rhs=mt[:, col // P, :],
                                         start=(j == 0), stop=(j == ncht - 1))
                        ci += 1
                    # mean scale on Act (per-partition invc), f32 out
                    aggm = fp.tile([P, f], f32, tag="aggm")
                    if ncht == 0:
                        nc.vector.memset(aggm[:], 0.0)
                    else:
                        nc.scalar.activation(out=aggm[:], in_=pt[:],
                                             func=mybir.ActivationFunctionType.Copy,
                                             scale=invc_t[:, t:t + 1])
                    paggT = psum_aux.tile([P, P], f32, tag="aux")
                    nc.tensor.transpose(out=paggT[:], in_=aggm[:], identity=ident_t[:])
                    aggT = fp.tile([P, P], bf16, tag="aggT_sb")
                    nc.scalar.activation(out=aggT[:], in_=paggT[:],
                                         func=mybir.ActivationFunctionType.Copy)
                    phT = psum_h.tile([P, P], f32, tag="hT")
                    nc.tensor.matmul(out=phT[:], lhsT=wl1_t[:], rhs=aggT[:],
                                     start=True, stop=False)
                    nc.tensor.matmul(out=phT[:], lhsT=wr1_t[:],
                                     rhs=xT_t[:, t * P:(t + 1) * P],
                                     start=False, stop=True)
                    hT_sl = hT_store[:, t * P:(t + 1) * P]
                    nc.scalar.activation(out=hT_sl, in_=phT[:],
                                         func=mybir.ActivationFunctionType.Relu,
                                         bias=b1_t[:], scale=1.0)
                    pz = psum_aux.tile([P, P], f32, tag="aux")
                    nc.tensor.matmul(out=pz[:, 0:64], lhsT=hT_sl, rhs=wl2p_t[:],
                                     start=True, stop=True)
                    nc.scalar.activation(out=z_acc[:, t, :], in_=pz[:, 0:64],
                                         func=mybir.ActivationFunctionType.Copy)

                    t_done += 1
                    if t_done in regions:
                        k = regions.index(t_done)
                        r0, r1 = rb_rows[k], rb_rows[k + 1]
                        t0 = rb_tiles[k]
                        # z_acc[[par half], t0:t, :] -> z2_own rows, col half
                        for par, pbase in ((0, 0), (1, 64)):
                            src_ap = z_acc[pbase:pbase + 64, t0:t_done, :]
                            dst_ap = z2_own[r0:r1, pbase:pbase + 64]
                            # reorder dst dims to (j, t, c) to match sbuf (part, t, c)
                            d3 = dst_ap.rearrange("(t j) c -> j t c", j=64)
                            nc.sync.dma_start(d3, src_ap)
                        if do_ag:
                            nc.gpsimd.collective_compute(
                                "AllGather", mybir.AluOpType.bypass,
                                replica_groups=[list(range(ncores))],
                                ins=[z2_own[r0:r1, :]],
                                outs=[z2_fullR[k][:, :, :]])

                # ================= LAYER 2 =================
                if not do_l2:
                    continue
                z_tabs = [z2_fullR[r][:, :, :].flatten_outer_dims() for r in range(nreg)]
                msg_bufs2 = {}

                qrot = [0]

                def ensure_win2(r, w):
                    if (r, w) in msg_bufs2:
                        return msg_bufs2[(r, w)]
                    lo = w * win
                    cnt = min(win, SR[r] - lo)
                    mt = mp2.tile([P, win // P, P], bf16, tag="m2")
                    nc.gpsimd.dma_gather(
                        mt[:, :cnt // P, :], z_tabs[r],
                        idxR_t[r][:, lo // 16:(lo + cnt) // 16], cnt, cnt, P,
                        queue_num=qrot[0] % 4, single_packet=single_packet)
                    qrot[0] += 1
                    msg_bufs2[(r, w)] = mt
                    return mt

                ci = 0
                for t in range(nt):
                    ncht = int(budget[t, :].sum())
                    pt = psum_seg.tile([P, f], f32, tag="seg")
                    for j in range(ncht):
                        r = int(chunk_reg[ci])
                        w, col = divmod(int(chunk_sci[ci]) * P, win)
                        mt = ensure_win2(r, w)
                        pbase = 64 * int(chunk_par[ci])
                        nc.tensor.matmul(out=pt[:, 0:ncl], lhsT=oh_t[:, ci, :],
                                         rhs=mt[:, col // P, pbase:pbase + ncl],
                                         start=(j == 0), stop=(j == ncht - 1))
                        ci += 1
                    s2 = fp.tile([P, ncl], f32, tag="s2")
                    if ncht == 0:
                        nc.vector.memset(s2[:], 0.0)
                    else:
                        nc.scalar.activation(out=s2[:], in_=pt[:, 0:ncl],
                                             func=mybir.ActivationFunctionType.Copy,
                                             scale=invc_t[:, t:t + 1])
                    po = psum_aux.tile([P, P], f32, tag="aux")
                    nc.tensor.matmul(out=po[:, 0:ncl], lhsT=hT_store[:, t * P:(t + 1) * P],
                                     rhs=wr2_t[:], start=True, stop=True)
                    ofin = out_acc[:, t, :]
                    nc.vector.tensor_add(out=ofin, in0=po[:, 0:ncl], in1=s2[:])
                    nc.vector.tensor_add(out=ofin, in0=ofin, in1=b2_t[:])

                # output write: per-tile strided (even/odd local) rows
                for t in range(nt):
                    rows = min(P, npc - t * P)
                    n_even = (rows + 1) // 2
                    n_odd = rows // 2
                    ev = out_acc[0:n_even, t, :]
                    od = out_acc[64:64 + n_odd, t, :]
                    dst_e = out_d[t * P:t * P + 2 * n_even - 1:2, :]
                    nc.sync.dma_start(dst_e, ev)
                    if n_odd:
                        dst_o = out_d[t * P + 1:t * P + 2 * n_odd:2, :]
                        nc.sync.dma_start(dst_o, od)

    nc.compile()
    return nc


import jax
from jax.sharding import Mesh, PartitionSpec
from jax.experimental.shard_map import shard_map
from concourse.bass2jax import _bass_exec_p, partition_id_tensor, install_neuronx_cc_hook


class SpmdRunner:
    def __init__(self, nc, n_cores: int):
        install_neuronx_cc_hook()
        self.nc = nc
        self.n_cores = n_cores
        partition_name = nc.partition_id_tensor.name if nc.partition_id_tensor else None
        in_names, out_names, out_avals = [], [], []
        zero_outs = []
        for alloc in nc.m.functions[0].allocations:
            if not isinstance(alloc, mybir.MemoryLocationSet):
                continue
            name = alloc.memorylocations[0].name
            if alloc.kind == "ExternalInput":
                if name != partition_name:
                    in_names.append(name)
            elif alloc.kind == "ExternalOutput":
                shape = tuple(alloc.tensor_shape)
                dtype = mybir.dt.np(alloc.dtype)
                out_names.append(name)
                out_avals.append(jax.core.ShapedArray(shape, dtype))
                zero_outs.append(np.zeros(shape, dtype))
        self.in_names = list(in_names)
        self.out_names = out_names
        self.out_avals = out_avals
        self.zero_outs = zero_outs
        n_params = len(in_names)
        all_in_names = list(in_names) + list(out_names)
        if partition_name is not None:
            all_in_names.append(partition_name)

        def _body(*args):
            operands = list(args)
            if partition_name is not None:
                operands.append(partition_id_tensor())
            outs = _bass_exec_p.bind(
                *operands,
                out_avals=tuple(out_avals),
                in_names=tuple(all_in_names),
                out_names=tuple(out_names),
                lowering_input_output_aliases=(),
                sim_require_finite=False,
                sim_require_nnan=False,
                nc=nc,
            )
            return tuple(outs)

        devices = jax.devices()[:n_cores]
        assert len(devices) == n_cores
        self.mesh = Mesh(np.asarray(devices), ("core",))
        in_specs = (PartitionSpec("core"),) * (n_params + len(out_names))
        out_specs = (PartitionSpec("core"),) * len(out_names)
        self.fn = jax.jit(
            shard_map(_body, mesh=self.mesh, in_specs=in_specs,
                      out_specs=out_specs, check_rep=False),
            keep_unused=True,
        )
        self._dev_args = None

    def stage(self, in_maps):
        n = self.n_cores
        concat_in = [
            np.concatenate([np.asarray(in_maps[c][name]) for c in range(n)], axis=0)
            for name in self.in_names
        ]
        concat_zeros = [
            np.zeros((n * z.shape[0], *z.shape[1:]), z.dtype) for z in self.zero_outs
        ]
        from jax.sharding import NamedSharding
        sh = NamedSharding(self.mesh, PartitionSpec("core"))
        self._dev_args = [jax.device_put(a, sh) for a in concat_in + concat_zeros]
        return self

    def run(self):
        return self.fn(*self._dev_args)

    def run_blocking(self):
        out = self.fn(*self._dev_args)
        jax.block_until_ready(out)
        return out

    def results(self, out_arrs):
        n = self.n_cores
        return [
            {name: np.asarray(out_arrs[i]).reshape(n, *self.out_avals[i].shape)[c]
             for i, name in enumerate(self.out_names)}
            for c in range(n)
        ]


# ---------------- self-contained entry point ----------------
_CACHE = {}

def kernel(**inputs):
    import numpy as _np
    x = _np.asarray(inputs["x"], dtype=_np.float32)
    edge_index = _np.asarray(inputs["edge_index"])
    Wl1 = _np.asarray(inputs["Wl1"], dtype=_np.float32)
    Wr1 = _np.asarray(inputs["Wr1"], dtype=_np.float32)
    b1 = _np.asarray(inputs["b1"], dtype=_np.float32)
    Wl2 = _np.asarray(inputs["Wl2"], dtype=_np.float32)
    Wr2 = _np.asarray(inputs["Wr2"], dtype=_np.float32)
    b2 = _np.asarray(inputs["b2"], dtype=_np.float32)
    N, F = x.shape
    H = Wl1.shape[1]
    C = Wl2.shape[1]
    import hashlib
    eh = hashlib.md5(edge_index.tobytes()).hexdigest()
    key = ("plan", N, F, H, C, edge_index.shape[1], eh)
    if key not in _CACHE:
        plan = make_plan(edge_index, N, F, H, C, 8)
        nc = build_program(plan)
        runner = SpmdRunner(nc, 8)
        _CACHE[key] = (plan, runner)
    plan, runner = _CACHE[key]
    in_maps = stage_inputs(plan, x, Wl1, Wr1, b1, Wl2, Wr2, b2)
    runner.stage(in_maps)
    out_arrs = runner.run_blocking()
    results = runner.results(out_arrs)
    out = _np.concatenate([results[c]["out"] for c in range(8)], axis=0)
    return out[:N].astype(_np.float32)

